# revision 1
# baseline (speedup 1.0000x reference)
"""DGCNN (nn_DGCNN_43911745634410) Trainium2 kernel.

Structure of the model: the only heavy compute is xw = x @ gcn1_W with
x [129, 262144] f32 (~135 MB) and gcn1_W [262144, 1] — a memory-bound matvec.
xw is shared by all three edge-attr channels (it does not depend on edge
weights). Everything downstream (segment-sums over 16K edges, a 129-element
sort, two tiny conv1ds and three FCs) is a few hundred KFLOPs.

Device strategy (8 NeuronCores, tensor-parallel over the feature dim F):
  - core c gets x[:, c*32768:(c+1)*32768] (16.5 MB) and the matching w slice;
  - a raw-Bass kernel streams the shard through SBUF and uses the DVE's fused
    scalar_tensor_tensor (out=(x*1)*w, accum_out=free-dim sum) to produce
    per-partition partial dot products at one DVE pass per element, so the
    kernel runs at the HBM/DMA roofline (~47 us per core);
  - bulk tiles are [128, 1024] (4 rows x 32 partitions-per-row), the last row
    is one short [128, 256] tile so the non-overlapped tail op is short.
  - partials ([128, 33] per core) are summed on the host in f64 (all-reduce
    across cores), and the tiny downstream runs on the host in f64, exactly
    matching the reference semantics (stable descending sort, PyG GCN
    normalization with self-loops, VALID conv1d/maxpool, ELU MLP).

The raw-Bass (no TileContext) form is deliberate: this toolchain encodes at
most ONE semaphore wait per instruction, so each x tile gets a dedicated SBUF
buffer (the whole shard fits: ~132 KB/partition of the 224 KB) and every wait
is a single explicit wait_ge.
"""
from contextlib import ExitStack

import numpy as np

import concourse.bass as bass
from concourse import mybir
from concourse.bass_utils import run_bass_kernel_spmd

F32 = mybir.dt.float32

N = 129
F = 262144
NCORES = 8
SH = F // NCORES          # 32768 features per core
FD = 1024                 # free elems per partition per bulk tile
PPR = SH // FD            # partitions per row = 32
RPT = 128 // PPR          # rows per bulk tile = 4
NB = 26                   # bulk tiles [128, 1024], rows 0..103
TFD = SH // 128           # 256: small-tile free dim (one row per tile)
NS = 25                   # small tiles [128, 256], rows 104..128
NCOL = NB + NS            # 48 partial columns

_NC_CACHE = None


def _build_matvec_bass():
    nc = bass.Bass("TRN2")
    x = nc.dram_tensor("x_s", [N * SH], F32, kind="ExternalInput")
    w = nc.dram_tensor("w_s", [SH], F32, kind="ExternalInput")
    sel = nc.dram_tensor("sel", [32, 259], F32, kind="ExternalInput")
    out = nc.dram_tensor("part", [128, NCOL], F32, kind="ExternalOutput")

    with ExitStack() as ctx:
        selt = ctx.enter_context(nc.sbuf_tensor("selt", [32, 259], F32))
        wq = ctx.enter_context(nc.sbuf_tensor("wq", [32, FD], F32))
        wnt = ctx.enter_context(nc.sbuf_tensor("wnt", [128, TFD], F32))
        wt_ps = ctx.enter_context(nc.psum_tensor("wt_ps", [128, FD], F32))
        wn_ps = ctx.enter_context(nc.psum_tensor("wn_ps", [128, TFD], F32))
        wt_sb = ctx.enter_context(nc.sbuf_tensor("wt_sb", [128, FD], F32))
        xts = [
            ctx.enter_context(nc.sbuf_tensor(f"xt{t}", [128, FD], F32))
            for t in range(NB)
        ]
        xss = [
            ctx.enter_context(nc.sbuf_tensor(f"xs{s}", [128, TFD], F32))
            for s in range(NS)
        ]
        part = ctx.enter_context(nc.sbuf_tensor("part_sb", [128, NCOL], F32))
        w_sem = ctx.enter_context(nc.semaphore("w_sem"))
        pe_sem = ctx.enter_context(nc.semaphore("pe_sem"))
        act_sem = ctx.enter_context(nc.semaphore("act_sem"))
        x_sems = [ctx.enter_context(nc.semaphore(f"x_sem{t}")) for t in range(NB)]
        s_sems = [ctx.enter_context(nc.semaphore(f"s_sem{s}")) for s in range(NS)]
        dve_sem = ctx.enter_context(nc.semaphore("dve_sem"))
        out_sem = ctx.enter_context(nc.semaphore("out_sem"))
        block = ctx.enter_context(nc.Block())

        base = NB * 128 * FD

        @block.sync
        def _(sync):
            # x0 first: its 1.5us transfer hides the descriptor-gen of the
            # three tiny w/i32 loads (gen cadence ~650ns/DMA would otherwise
            # put ~1.1us of gaps at the stream head).
            src0 = x[0 : 128 * FD].rearrange("(p f) -> p f", f=FD)
            sync.dma_start(xts[0][:, :], src0).then_inc(x_sems[0], 16)
            sync.dma_start(selt[:, :], sel[:, :]).then_inc(w_sem, 16)
            sync.dma_start(
                wq[:, :], w[:].rearrange("(q j) -> q j", j=FD)
            ).then_inc(w_sem, 16)
            for t in range(1, NB):
                src = x[t * 128 * FD : (t + 1) * 128 * FD].rearrange(
                    "(p f) -> p f", f=FD
                )
                sync.dma_start(xts[t][:, :], src).then_inc(x_sems[t], 16)
            for s in range(NS):
                src = x[base + s * 128 * TFD : base + (s + 1) * 128 * TFD].rearrange(
                    "(p f) -> p f", f=TFD
                )
                sync.dma_start(xss[s][:, :], src).then_inc(s_sems[s], 16)
            sync.wait_ge(dve_sem, NCOL)
            sync.dma_start(out[:, :], part[:, :]).then_inc(out_sem, 16)

        @block.tensor
        def _(tensor):
            tensor.wait_ge(w_sem, 32)  # sel + wq loaded
            nc.tensor.matmul(
                wt_ps[:, 0:512], selt[:, 0:128], wq[:, 0:512],
                start=True, stop=True,
            ).then_inc(pe_sem, 1)
            nc.tensor.matmul(
                wt_ps[:, 512:FD], selt[:, 0:128], wq[:, 512:FD],
                start=True, stop=True,
            ).then_inc(pe_sem, 1)
            # wn_ps[p, i] = wq[p//4, (p%4)*256 + i]: four accumulating
            # matmuls; lhsT_b = iselt[:, 3-b : 131-b] has ones at (q, 4q+b),
            # so pass b contributes rows p%4 == b and exact zeros elsewhere.
            for b in range(4):
                nc.tensor.matmul(
                    wn_ps[:, :], selt[:, 131 - b : 259 - b],
                    wq[:, b * TFD : (b + 1) * TFD],
                    start=(b == 0), stop=(b == 3),
                ).then_inc(pe_sem, 1)

        @block.scalar
        def _(scalar):
            scalar.wait_ge(pe_sem, 2)
            nc.scalar.copy(wt_sb[:, :], wt_ps[:, :]).then_inc(act_sem, 1)
            scalar.wait_ge(pe_sem, 6)
            nc.scalar.copy(wnt[:, :], wn_ps[:, :]).then_inc(act_sem, 1)

        @block.vector
        def _(vector):
            vector.wait_ge(act_sem, 1)
            for t in range(NB):
                vector.wait_ge(x_sems[t], 16)
                nc.vector.scalar_tensor_tensor(
                    xts[t][:, :],
                    xts[t][:, :],
                    1.0,
                    wt_sb[:, :],
                    op0=mybir.AluOpType.mult,
                    op1=mybir.AluOpType.mult,
                    accum_out=part[:, t : t + 1],
                ).then_inc(dve_sem, 1)
            vector.wait_ge(act_sem, 2)
            for s in range(NS):
                vector.wait_ge(s_sems[s], 16)
                nc.vector.scalar_tensor_tensor(
                    xss[s][:, :],
                    xss[s][:, :],
                    1.0,
                    wnt[:, :],
                    op0=mybir.AluOpType.mult,
                    op1=mybir.AluOpType.mult,
                    accum_out=part[:, NB + s : NB + s + 1],
                ).then_inc(dve_sem, 1)

    return nc



def get_matvec_bass():
    global _NC_CACHE
    if _NC_CACHE is None:
        _NC_CACHE = _build_matvec_bass()
    return _NC_CACHE


def _make_core_inputs(x_np, w_np, core):
    xs = np.ascontiguousarray(x_np[:, core * SH : (core + 1) * SH]).reshape(-1)
    ws = np.ascontiguousarray(w_np[core * SH : (core + 1) * SH])
    sel = np.zeros((32, 259), np.float32)
    sel[:, 0:128] = np.tile(np.eye(32, dtype=np.float32), (1, 4))
    sel[np.arange(32), 131 + 4 * np.arange(32)] = 1.0
    return {"x_s": xs, "w_s": ws, "sel": sel}


def _reduce_parts(parts):
    """parts: 8 arrays [128, NCOL] f32 -> xw [N] f64."""
    xw = np.zeros(N, np.float64)
    for part in parts:
        p = part.astype(np.float64)
        for t in range(NB):
            xw[RPT * t : RPT * (t + 1)] += p[:, t].reshape(RPT, PPR).sum(1)
        for si in range(NS):
            xw[RPT * NB + si] += p[:, NB + si].sum()
    return xw


def _matvec_device(x_np, w_np):
    """x [N, F] f32, w [F] f32 -> xw [N] f64 via the 8-core bass kernel."""
    global _NC_CACHE
    in_maps = [_make_core_inputs(x_np, w_np, c) for c in range(NCORES)]
    last_exc = None
    for attempt in range(2):
        try:
            nc = get_matvec_bass()
            res = run_bass_kernel_spmd(nc, in_maps, core_ids=list(range(NCORES)))
            return _reduce_parts([res.results[c]["part"] for c in range(NCORES)])
        except Exception as e:  # transient NRT_EXEC_UNIT_UNRECOVERABLE seen once
            import sys

            print(f"kernel: device run attempt {attempt} failed: {e!r:.200}",
                  file=sys.stderr)
            last_exc = e
            _NC_CACHE = None
    # Last-resort host fallback so a transient device failure still yields a
    # correct result (numerically equivalent partial-sum structure).
    import sys

    print(f"kernel: device path failed twice ({last_exc!r:.200}); "
          "falling back to host matvec", file=sys.stderr)
    prod = x_np.astype(np.float64) * w_np.astype(np.float64)[None, :]
    return prod.sum(axis=1)


def _downstream(xw, inputs):
    """Everything after xw = x @ gcn1_W, in f64 numpy. Returns [1, 2] f32."""
    edge_index = np.asarray(inputs["edge_index"]).astype(np.int64)
    row, col = edge_index[0], edge_index[1]
    edge_attr = np.asarray(inputs["edge_attr"], np.float64)
    g1b = np.asarray(inputs["gcn1_b"], np.float64)
    g2W = np.asarray(inputs["gcn2_W"], np.float64)
    g2b = np.asarray(inputs["gcn2_b"], np.float64)
    c1w = np.asarray(inputs["conv1_w"], np.float64)
    c1b = np.asarray(inputs["conv1_b"], np.float64)
    c2w = np.asarray(inputs["conv2_w"], np.float64)
    c2b = np.asarray(inputs["conv2_b"], np.float64)
    f1W = np.asarray(inputs["fc1_W"], np.float64)
    f1b = np.asarray(inputs["fc1_b"], np.float64)
    f2W = np.asarray(inputs["fc2_W"], np.float64)
    f2b = np.asarray(inputs["fc2_b"], np.float64)
    f3W = np.asarray(inputs["fc3_W"], np.float64)
    f3b = np.asarray(inputs["fc3_b"], np.float64)

    n = N
    loop = np.arange(n)
    row2 = np.concatenate([row, loop])
    col2 = np.concatenate([col, loop])

    def gcn(xw_vec, ew):
        # PyG GCNConv with edge weights: self-loops (weight 1), symmetric norm.
        ew2 = np.concatenate([ew, np.ones(n)])
        deg = np.zeros(n)
        np.add.at(deg, col2, ew2)
        dinv = np.where(deg > 0, deg**-0.5, 0.0)
        norm = dinv[row2] * ew2 * dinv[col2]
        out = np.zeros(n)
        np.add.at(out, col2, norm * xw_vec[row2])
        return out

    outs = []
    for c in range(3):
        ew = edge_attr[:, c]
        h1 = gcn(xw, ew) + g1b[0]
        h2 = gcn(h1 * g2W[0, 0], ew) + g2b[0]
        # SortPool: jnp.argsort(-h2) is a stable ascending sort of the negation
        perm = np.argsort(-h2, kind="stable")
        hs = np.stack([h1[perm], h2[perm]], axis=1)  # [n, 2]
        z = hs.T  # [2, n]
        L = z.shape[1] - 2
        z1 = np.zeros((3, L))
        for o in range(3):
            for i in range(2):
                for k in range(3):
                    z1[o] += c1w[o, i, k] * z[i, k : k + L]
            z1[o] += c1b[o]
        z1p = np.max(np.stack([z1[:, 0 : L - 2], z1[:, 1 : L - 1], z1[:, 2:L]], 0), 0)
        L2 = z1p.shape[1] - 2
        z2 = np.zeros((1, L2))
        for i in range(3):
            for k in range(3):
                z2[0] += c2w[0, i, k] * z1p[i, k : k + L2]
        z2[0] += c2b[0]
        z2p = np.max(
            np.stack([z2[:, 0 : L2 - 2], z2[:, 1 : L2 - 1], z2[:, 2:L2]], 0), 0
        )
        outs.append(z2p)  # [1, 121]

    allx = np.concatenate(outs, axis=0)  # [3, 121]
    h = allx.reshape(1, -1)

    def elu(v):
        return np.where(v > 0, v, np.expm1(v))

    h = elu(h @ f1W + f1b)
    h = elu(h @ f2W + f2b)
    out = h @ f3W + f3b
    return out.astype(np.float32)


def kernel(**inputs) -> np.ndarray:
    x = np.ascontiguousarray(np.asarray(inputs["x"], np.float32))
    w = np.asarray(inputs["gcn1_W"], np.float32).reshape(-1)
    xw = _matvec_device(x, w)
    return _downstream(xw, inputs)



# revision 17
# speedup vs baseline: 1.6757x; 1.6757x over previous
"""DGCNN (nn_DGCNN_43911745634410) Trainium2 kernel — bf16 streaming matvec.

The only heavy compute is xw = x @ gcn1_W with x [129, 262144] f32 (~135 MB)
— a memory-bound matvec whose result feeds a tiny host-side GCN/sort/conv
head. Device strategy (8 cores, tensor-parallel over F):

  - core c owns x[:, c*32768:(c+1)*32768]. The host converts the shard to
    bfloat16 (the correctness budget is rel_err < 2e-2; bf16 quantization
    contributes ~3e-3 end to end), HALVING the HBM traffic that bounds this
    kernel: ~8.5 MB/core streams at the 360 GB/s DMA roofline in ~24 us
    (vs ~47.5 us for f32).
  - the DVE cannot run its fused multiply+accumulate (scalar_tensor_tensor)
    in a fast mode, so the multiply and the reduction are split:
      * multiply: tensor_tensor (2x mode for 2-byte dtypes, 0.52 ns/elem)
        in place over the shard, weights pre-replicated per partition;
      * reduction to per-partition partials: spread across THREE engines so
        no engine exceeds the DMA budget — DVE tensor_scalar (4x mode,
        0.26 ns/elem), Act activation-with-accumulator (0.83 ns/elem), and
        Pool/GPSIMD tensor_scalar (~1.4 ns/elem).
  - tiles are [128, 1024]: 4 x-rows per tile, 32 partitions per row; row 128
    rides as a [32, 1024] tile reusing wtb[0:32], streamed EARLY so its slow
    Pool reduction overlaps the stream; tile 31 is split [128,768]+[128,256]
    so the last item in the DMA stream has a short mult+reduce tail on DVE.
  - semaphore increments are batched (one s_m inc per arrival group) because
    each then_inc costs ~70 ns of DVE sequencer time.
  - one [128, 34] f32 partial block per core returns to the host, which
    all-reduces in f64 and runs the tiny downstream exactly matching the
    reference semantics.

Raw Bass (no TileContext): this toolchain encodes at most ONE semaphore wait
per instruction, so cross-engine ordering uses counting semaphores (s_m =
DVE mult progress, s_done = reductions completed) plus per-group DMA sems.
"""
from contextlib import ExitStack

import numpy as np
import ml_dtypes

import concourse.bass as bass
from concourse import mybir
from concourse.bass_utils import run_bass_kernel_spmd

F32 = mybir.dt.float32
DT = mybir.dt.bfloat16
NPDT = ml_dtypes.bfloat16

N = 129
F = 262144
NCORES = 8
SH = F // NCORES          # 32768 features per core
FD = 1024                 # free elems per partition per tile
PPR = SH // FD            # partitions per row = 32
RPT = 128 // PPR          # rows per bulk tile = 4
NBT = 32                  # bulk tiles (rows 0..127); tile 31 split 768+256

# s_m increment groups: each mult waits only for ITS OWN tile's DMA (so the
# multiply pipeline tracks arrivals), but s_m is bumped once per group (a
# then_inc costs ~70 ns of DVE sequencer time). "xr" = row-128 tile,
# "A"/"B" = tile-31 split.
GROUPS = [
    ("xr",), (0,), (1, 2), (3, 4, 5, 6), (7, 8, 9, 10), (11, 12, 13, 14),
    (15, 16, 17, 18), (19, 20, 21, 22), (23, 24, 25, 26), (27,), (28,),
    (29,), (30,), ("A",), ("B",),
]
# s_m value after group g's mults complete = g+1
_READY = {}
for _g, _tiles in enumerate(GROUPS):
    for _t in _tiles:
        _READY[_t] = _g + 1

# Reduction-engine assignment, rate-matched: tiles release in groups of 4
# every ~2.9us; Act absorbs ~2.4 ops/window (1.22us/op), Pool ~1.9 (1.52us),
# DVE has no mid-stream slack (mult chain tracks arrivals) so it only takes
# the post-stream tail.
DVE_TILES = [28, 29, 30]               # + "A" reduced on DVE at the tail
ACT_TILES = [0, 1, 3, 4, 7, 8, 11, 12, 15, 16, 19, 20, 23, 24, 27]
POOL_TILES = [2, 5, 6, 9, 10, 13, 14, 17, 18, 21, 22, 25, 26]  # + "xr", "B"
# Partial-column layout (order of accum_out columns in parts[128, NCOL])
COLS = (
    [("b", t) for t in DVE_TILES] + [("A",)]
    + [("b", t) for t in ACT_TILES]
    + [("b", t) for t in POOL_TILES] + [("xr",), ("B",)]
)
NCOL = len(COLS)          # 34

_NC_CACHE = None


def _build_matvec_bass():
    nc = bass.Bass("TRN2")
    x = nc.dram_tensor("x_s", [N * SH], DT, kind="ExternalInput")
    wt = nc.dram_tensor("wt_s", [128, FD], DT, kind="ExternalInput")
    out = nc.dram_tensor("parts", [128, NCOL], F32, kind="ExternalOutput")

    col_of = {c: i for i, c in enumerate(COLS)}

    with ExitStack() as ctx:
        xbuf = ctx.enter_context(nc.sbuf_tensor("xbuf", [128, NBT * FD], DT))
        xrow = ctx.enter_context(nc.sbuf_tensor("xrow", [32, FD], DT))
        wtb = ctx.enter_context(nc.sbuf_tensor("wtb", [128, FD], DT))
        scr_d = ctx.enter_context(nc.sbuf_tensor("scr_d", [128, FD], DT))
        scr_a = ctx.enter_context(nc.sbuf_tensor("scr_a", [128, FD], DT))
        scr_p = ctx.enter_context(nc.sbuf_tensor("scr_p", [128, FD], DT))
        parts = ctx.enter_context(nc.sbuf_tensor("parts_sb", [128, NCOL], F32))
        all_tiles = [t for g in GROUPS for t in g]
        t_sems = {
            t: ctx.enter_context(nc.semaphore(f"s_t{t}")) for t in all_tiles
        }
        s_m = ctx.enter_context(nc.semaphore("s_m"))
        s_done = ctx.enter_context(nc.semaphore("s_done"))
        s_out = ctx.enter_context(nc.semaphore("s_out"))
        block = ctx.enter_context(nc.Block())

        # per-tile sem thresholds: 16 per DMA (+16 for wt, which also bumps
        # the xrow sem so the first mult's single wait covers both)
        thr = {t: 16 for t in all_tiles}
        thr["xr"] += 16

        def tile_src(t):
            return x[t * 128 * FD : (t + 1) * 128 * FD].rearrange(
                "(p f) -> p f", f=FD
            )

        @block.sync
        def _(sync):
            # wt first (every mult needs it; m0 waits on wt+x0 via one sem,
            # later DVE ops are engine-ordered behind m0). Row 128 second so
            # its slow Pool reduction overlaps the bulk stream.
            sync.dma_start(wtb[:, :], wt[:, :]).then_inc(t_sems["xr"], 16)
            srcr = x[128 * SH : 128 * SH + 32 * FD].rearrange("(p f) -> p f", f=FD)
            sync.dma_start(xrow[:, :], srcr).then_inc(t_sems["xr"], 16)
            for t in range(NBT - 1):
                sync.dma_start(
                    xbuf[:, t * FD : (t + 1) * FD], tile_src(t)
                ).then_inc(t_sems[t], 16)
            # tile 31 split 768 + 256 so the stream tail is short
            t31 = tile_src(31)
            sync.dma_start(
                xbuf[:, 31 * FD : 31 * FD + 768], t31[:, 0:768]
            ).then_inc(t_sems["A"], 16)
            sync.dma_start(
                xbuf[:, 31 * FD + 768 : 32 * FD], t31[:, 768:FD]
            ).then_inc(t_sems["B"], 16)
            sync.wait_ge(s_done, NCOL)
            sync.dma_start(out[:, :], parts[:, :]).then_inc(s_out, 16)

        def red_dve(src_ap, scr_ap, col):
            nc.vector.tensor_scalar(
                scr_ap, src_ap, 1.0, None, mybir.AluOpType.mult,
                accum_out=parts[:, col : col + 1],
            ).then_inc(s_done, 1)

        @block.vector
        def _(vector):
            nc.vector.memset(parts[:, :], 0.0)

            def ap_of(t):
                if t == "xr":
                    return xrow[:, :]
                if t == "A":
                    return xbuf[:, 31 * FD : 31 * FD + 768]
                if t == "B":
                    return xbuf[:, 31 * FD + 768 : 32 * FD]
                return xbuf[:, t * FD : (t + 1) * FD]

            def wt_of(t):
                if t == "xr":
                    return wtb[0:32, :]
                if t == "A":
                    return wtb[:, 0:768]
                if t == "B":
                    return wtb[:, 768:FD]
                return wtb[:, :]

            # DVE has no mid-stream slack (mult chain tracks DMA arrivals at
            # ~100% utilization), so its reduces sit in the post-stream tail.
            for g, tiles in enumerate(GROUPS):
                for i, t in enumerate(tiles):
                    vector.wait_ge(t_sems[t], thr[t])
                    ins = nc.vector.tensor_tensor(
                        ap_of(t), ap_of(t), wt_of(t), mybir.AluOpType.mult
                    )
                    if i == len(tiles) - 1:
                        ins.then_inc(s_m, 1)
                if tiles == (29,):
                    red_dve(ap_of(28), scr_d[:, :], col_of[("b", 28)])
            red_dve(ap_of(29), scr_d[:, :], col_of[("b", 29)])
            red_dve(ap_of(30), scr_d[:, :], col_of[("b", 30)])
            red_dve(ap_of("A"), scr_d[:, 0:768], col_of[("A",)])

        @block.scalar
        def _(scalar):
            for t in ACT_TILES:
                scalar.wait_ge(s_m, _READY[t])
                nc.scalar.activation(
                    scr_a[:, :],
                    xbuf[:, t * FD : (t + 1) * FD],
                    mybir.ActivationFunctionType.Copy,
                    accum_out=parts[:, col_of[("b", t)] : col_of[("b", t)] + 1],
                ).then_inc(s_done, 1)

        @block.gpsimd
        def _(gpsimd):
            done_xr = False
            for t in POOL_TILES:
                if not done_xr and _READY[t] > _READY["xr"]:
                    gpsimd.wait_ge(s_m, _READY["xr"])
                    c = col_of[("xr",)]
                    nc.gpsimd.tensor_scalar(
                        scr_p[0:32, :], xrow[:, :], 1.0, None,
                        mybir.AluOpType.mult,
                        accum_out=parts[0:32, c : c + 1],
                    ).then_inc(s_done, 1)
                    done_xr = True
                gpsimd.wait_ge(s_m, _READY[t])
                nc.gpsimd.tensor_scalar(
                    scr_p[:, :],
                    xbuf[:, t * FD : (t + 1) * FD],
                    1.0, None, mybir.AluOpType.mult,
                    accum_out=parts[:, col_of[("b", t)] : col_of[("b", t)] + 1],
                ).then_inc(s_done, 1)
            # B's product lands last; Pool is free by then and a [128, 256]
            # op costs it only ~450 ns, relieving the DVE tail pile.
            gpsimd.wait_ge(s_m, _READY["B"])
            cB = col_of[("B",)]
            nc.gpsimd.tensor_scalar(
                scr_p[:, 0:256],
                xbuf[:, 31 * FD + 768 : 32 * FD],
                1.0, None, mybir.AluOpType.mult,
                accum_out=parts[:, cB : cB + 1],
            ).then_inc(s_done, 1)

    return nc


def get_matvec_bass():
    global _NC_CACHE
    if _NC_CACHE is None:
        _NC_CACHE = _build_matvec_bass()
    return _NC_CACHE


def _make_core_inputs(x_np, w_np, core):
    xs = np.ascontiguousarray(x_np[:, core * SH : (core + 1) * SH])
    ws = w_np[core * SH : (core + 1) * SH]
    wt = np.tile(ws.reshape(PPR, FD), (RPT, 1))  # [128, FD]
    return {
        "x_s": xs.reshape(-1).astype(NPDT),
        "wt_s": wt.astype(NPDT),
    }


def _reduce_parts(parts_list):
    """parts_list: 8 arrays [128, NCOL] f32 -> xw [N] f64."""
    xw = np.zeros(N, np.float64)
    for part in parts_list:
        p = part.astype(np.float64)
        for c, col in enumerate(COLS):
            v = p[:, c]
            if col[0] == "b":
                t = col[1]
                xw[RPT * t : RPT * (t + 1)] += v.reshape(RPT, PPR).sum(1)
            elif col[0] in ("A", "B"):
                xw[124:128] += v.reshape(RPT, PPR).sum(1)
            else:  # ("xr",) row 128 lives on partitions 0..31
                xw[128] += v[0:32].sum()
    return xw


def _matvec_device(x_np, w_np):
    """x [N, F] f32, w [F] f32 -> xw [N] f64 via the 8-core bass kernel."""
    global _NC_CACHE
    in_maps = [_make_core_inputs(x_np, w_np, c) for c in range(NCORES)]
    last_exc = None
    for attempt in range(2):
        try:
            nc = get_matvec_bass()
            res = run_bass_kernel_spmd(nc, in_maps, core_ids=list(range(NCORES)))
            return _reduce_parts([res.results[c]["parts"] for c in range(NCORES)])
        except Exception as e:  # transient NRT failures seen historically
            import sys

            print(f"kernel: device run attempt {attempt} failed: {e!r:.200}",
                  file=sys.stderr)
            last_exc = e
            _NC_CACHE = None
    # Last-resort host fallback so a transient device failure still yields a
    # correct result.
    import sys

    print(f"kernel: device path failed twice ({last_exc!r:.200}); "
          "falling back to host matvec", file=sys.stderr)
    prod = x_np.astype(np.float64) * w_np.astype(np.float64)[None, :]
    return prod.sum(axis=1)


def _downstream(xw, inputs):
    """Everything after xw = x @ gcn1_W, in f64 numpy. Returns [1, 2] f32."""
    edge_index = np.asarray(inputs["edge_index"]).astype(np.int64)
    row, col = edge_index[0], edge_index[1]
    edge_attr = np.asarray(inputs["edge_attr"], np.float64)
    g1b = np.asarray(inputs["gcn1_b"], np.float64)
    g2W = np.asarray(inputs["gcn2_W"], np.float64)
    g2b = np.asarray(inputs["gcn2_b"], np.float64)
    c1w = np.asarray(inputs["conv1_w"], np.float64)
    c1b = np.asarray(inputs["conv1_b"], np.float64)
    c2w = np.asarray(inputs["conv2_w"], np.float64)
    c2b = np.asarray(inputs["conv2_b"], np.float64)
    f1W = np.asarray(inputs["fc1_W"], np.float64)
    f1b = np.asarray(inputs["fc1_b"], np.float64)
    f2W = np.asarray(inputs["fc2_W"], np.float64)
    f2b = np.asarray(inputs["fc2_b"], np.float64)
    f3W = np.asarray(inputs["fc3_W"], np.float64)
    f3b = np.asarray(inputs["fc3_b"], np.float64)

    n = N
    loop = np.arange(n)
    row2 = np.concatenate([row, loop])
    col2 = np.concatenate([col, loop])

    def gcn(xw_vec, ew):
        # PyG GCNConv with edge weights: self-loops (weight 1), symmetric norm.
        ew2 = np.concatenate([ew, np.ones(n)])
        deg = np.zeros(n)
        np.add.at(deg, col2, ew2)
        dinv = np.where(deg > 0, deg**-0.5, 0.0)
        norm = dinv[row2] * ew2 * dinv[col2]
        out = np.zeros(n)
        np.add.at(out, col2, norm * xw_vec[row2])
        return out

    outs = []
    for c in range(3):
        ew = edge_attr[:, c]
        h1 = gcn(xw, ew) + g1b[0]
        h2 = gcn(h1 * g2W[0, 0], ew) + g2b[0]
        # SortPool: jnp.argsort(-h2) is a stable ascending sort of the negation
        perm = np.argsort(-h2, kind="stable")
        hs = np.stack([h1[perm], h2[perm]], axis=1)  # [n, 2]
        z = hs.T  # [2, n]
        L = z.shape[1] - 2
        z1 = np.zeros((3, L))
        for o in range(3):
            for i in range(2):
                for k in range(3):
                    z1[o] += c1w[o, i, k] * z[i, k : k + L]
            z1[o] += c1b[o]
        z1p = np.max(np.stack([z1[:, 0 : L - 2], z1[:, 1 : L - 1], z1[:, 2:L]], 0), 0)
        L2 = z1p.shape[1] - 2
        z2 = np.zeros((1, L2))
        for i in range(3):
            for k in range(3):
                z2[0] += c2w[0, i, k] * z1p[i, k : k + L2]
        z2[0] += c2b[0]
        z2p = np.max(
            np.stack([z2[:, 0 : L2 - 2], z2[:, 1 : L2 - 1], z2[:, 2:L2]], 0), 0
        )
        outs.append(z2p)  # [1, 121]

    allx = np.concatenate(outs, axis=0)  # [3, 121]
    h = allx.reshape(1, -1)

    def elu(v):
        return np.where(v > 0, v, np.expm1(v))

    h = elu(h @ f1W + f1b)
    h = elu(h @ f2W + f2b)
    out = h @ f3W + f3b
    return out.astype(np.float32)


_LAST_XW = None


def kernel(**inputs) -> np.ndarray:
    global _LAST_XW
    x = np.ascontiguousarray(np.asarray(inputs["x"], np.float32))
    w = np.asarray(inputs["gcn1_W"], np.float32).reshape(-1)
    xw = _matvec_device(x, w)
    _LAST_XW = xw
    return _downstream(xw, inputs)
